# revision 45
# baseline (speedup 1.0000x reference)
"""Single-head attention (B=4, S=4096, E=1024, D=64) on 8 TRN2 NeuronCores.

Sharding: data-parallel over (batch, query-half): core c handles batch
b = c // 2 and query rows [h*2048, (h+1)*2048) with h = c % 2. Each core
computes Q for its own 2048 rows and K/V for the full 4096 rows of its batch
(inputs are shipped host-pretransposed per half, so no duplicated DMA).

Per-core dataflow (TensorE matmuls in bf16 — fp32/fp32r matmuls run the PE
at half clock; fp32 accumulation in PSUM). Projections pack TWO weight
matrices into one 128-wide stationary operand:
  qk [128, 2048] = [K^T_own; Q^T_own]     (pass A, lhsT = [WkT | WqT])
  kv [128, 2048] = [K^T_oth; V^T_oth]     (pass B, lhsT = [WkT | WvT])
  vt [65, 2048]  = V^T_own + ones row      (pass C, lhsT = WvT)
Q^T / V^T_oth are then shifted to base partition 0 by SBUF-to-SBUF DMAs
(matmul operands must share a base partition).
  scores^T[k, q] = K^T.T @ Q^T -> exp -> P bf16
  attn^T[65, q] += V_aug.T @ P   (row 64 accumulates softmax denominators)
  output = attn^T with denominators; host transposes + normalizes.

The exp is split across two engines so ScalarE (1 elem/cycle/lane at
1.2 GHz, ~1.15us per [128,1024] tile) stops pacing the pipeline: 2/3 of
k-tiles get the exact ACT exp on ScalarE; every third tile is computed on
VectorE with a one-instruction Schraudolph bit-trick: i16 = rne(x*A + B)
reinterpreted as bf16 approximates exp(SCALE*x) (piecewise-linear mantissa
chord, ~1.8% log-error sawtooth whose mean bias cancels in the softmax
numerator/denominator ratio; applied to 20/64 of the weights it adds
~0.6% output rel err). The two engines use SEPARATE P-tile pools — a
shared pool serializes them via buffer-reuse ordering.

The attention runs as TWO passes over q (1024 columns each): the attn
accumulator then fits 2 PSUM banks, freeing a third scores slot (PSUM slot
contention paced the single-pass version), and pass 0's output ships
mid-kernel.

The HAM duty controller halves the PE clock after ~2.5us of PE idleness
and takes 5-13us at half clock to re-grant full duty, so the kernel keeps
the PE streaming: junk-fed 512-col warm-up matmuls run from the instant
the PE preamble ends until the first input DMAs land (gated only on a
VectorE memset, not on make_identity's gpsimd iota), and junk fillers
bridge the group-2 DMA wait at pass-0 iters 4-5. Input DMA is issued in
deadline order (wt + own q-cols 0:1024, then own 1024:2048, then the
other half) across the sync/scalar/gpsimd queues; the pass-0 side-slot
schedule (projection lumps A2/C2/A3/C3, then B0-B3) tracks the measured
arrival of those groups.
"""

import numpy as np

B, S, E, D = 4, 4096, 1024, 64
HALF = S // 2
N_CORES = 8
SCALE = 1.0 / np.sqrt(D)

NE = E // 128  # 8 e-tiles
NKT = S // 128  # 32 k-tiles
N_WARM = 11  # 512-col PE warm-up matmuls covering the preamble + DMA wait

# Schraudolph exp-approx constants (bf16 bit pattern via int16):
#   i16 = round(x * A16 + B16); bitcast bf16 ~= exp(SCALE * x)
LOG2E = 1.4426950408889634
A16 = SCALE * 128.0 * LOG2E
B16 = 127.0 * 128.0 - 7.3


_CACHE = {}


def _build():
    if "nc" in _CACHE:
        return _CACHE["nc"]

    from contextlib import ExitStack

    import concourse.bacc as bacc
    import concourse.tile as tile
    from concourse import mybir
    from concourse.masks import make_identity

    FP32 = mybir.dt.float32
    BF16 = mybir.dt.bfloat16
    I16 = mybir.dt.int16
    Exp = mybir.ActivationFunctionType.Exp
    Cpy = mybir.ActivationFunctionType.Copy
    Mult = mybir.AluOpType.mult
    Add = mybir.AluOpType.add

    nc = bacc.Bacc(
        "TRN2", target_bir_lowering=False, debug=False, num_devices=N_CORES
    )

    xt_q_d = nc.dram_tensor("xt_q", [E, HALF], BF16, kind="ExternalInput").ap()
    xt_o_d = nc.dram_tensor("xt_o", [E, HALF], BF16, kind="ExternalInput").ap()
    wt_d = nc.dram_tensor("wt", [E, 384], BF16, kind="ExternalInput").ap()
    out_d = nc.dram_tensor("out", [D + 1, HALF], BF16, kind="ExternalOutput").ap()

    with tile.TileContext(nc) as tc, ExitStack() as ctx:
        const = ctx.enter_context(tc.tile_pool(name="const", bufs=1))
        big = ctx.enter_context(tc.tile_pool(name="big", bufs=1))
        # P tiles: each [128,1024] tile gets its c0/c1 halves written by
        # DIFFERENT exp engines (balanced S/V split), so one pool is fine —
        # slot reuse gates both engines on the same attn read.
        pp = ctx.enter_context(tc.tile_pool(name="pp", bufs=7))
        # psA: six single-bank [128,512] score/side slots; psB: the 2-bank
        # attn accumulator. 6*2KB + 4KB = 16KB = all 8 PSUM banks.
        psA = ctx.enter_context(tc.tile_pool(name="psA", bufs=6, space="PSUM"))
        psB = ctx.enter_context(tc.tile_pool(name="psB", bufs=1, space="PSUM"))

        identB = const.tile([128, 128], BF16)
        junk = const.tile([128, 512], BF16)
        # memset on gpsimd: its preamble ends ~1us before vector's, so
        # the PE warm-up starts that much earlier.
        nc.gpsimd.memset(junk[:, :], 0.0)
        make_identity(nc, identB)

        xt = big.tile([128, NE, S], BF16)  # x^T; cols [0, HALF) = own q-rows
        wt = big.tile([128, NE, 384], BF16)  # [WkT|WqT | WkT|WvT | WvT | WqT]
        qk = big.tile([128, HALF], BF16)  # rows 0-63 K^T own, 64-127 Q^T own
        kv = big.tile([128, HALF], BF16)  # rows 0-63 K^T oth, 64-127 V^T oth
        qts = big.tile([64, HALF], BF16)  # Q^T shifted to base partition 0
        vto = big.tile([64, HALF], BF16)  # V^T other shifted to base part. 0
        vt = big.tile([65, HALF], BF16)  # V^T own; row 64 = ones
        vn = big.tile([128, NKT, D + 1], BF16)  # V natural + ones column
        att_sb = big.tile([65, HALF], BF16)  # attn^T + denominator row
        # odd K^T tiles shifted to base partition 64 for pass-1 row
        # tiling (rows 64-127: own tiles 1,3..15 / oth tiles 17,19..31)
        ko = big.tile([128, 1024], BF16)
        kq = big.tile([128, 1024], BF16)

        # --- PE warm-up: the HAM duty controller halves the PE clock after
        # ~2.5us of idleness and takes 5-13us at half clock to restore full
        # duty, so keep the PE streaming junk matmuls from the instant its
        # preamble ends until the first input DMAs land.
        warm = psA.tile([128, 512], FP32, tag="ps")
        for _ in range(N_WARM):
            nc.tensor.matmul(
                out=warm[0:128, 0:512],
                lhsT=junk[:, 0:128],
                rhs=junk[:, :],
                start=True,
                stop=True,
            )

        # --- input DMAs: ALL on the sync queue, issued in consumption
        # order. A single queue drains FIFO across all 16 DMA engines, so
        # arrival order == issue order and the PE never waits on a piece
        # that lost a queue-arbitration race (the old 3-queue split left
        # 12.6us of mid-ramp PE idle + HAM re-throttles). Measured stream
        # rate ~0.32 MB/us: wt ~9.5us, own 0:512 ~12.8, 512:1024 ~16.1,
        # own 1024:2048 ~22.7, oth chunks ~26/29/33/36us.
        xq_r = xt_q_d.rearrange("(t p) s -> p t s", p=128)
        xo_r = xt_o_d.rearrange("(t p) s -> p t s", p=128)
        wt_r = wt_d.rearrange("(t p) d -> p t d", p=128)
        # wt and own cols 0:512 interleaved per e-tile PAIR so the fused
        # A/C/Q prologue's et-pair k needs only the first 2k+2 pieces —
        # first useful matmul ~3us earlier than with a monolithic wt load.
        # Pieces stay >=192KB: below ~256KB the ~625ns-per-start HWDGE
        # config rate on sync throttles the stream under the ~0.36MB/us
        # the HBM path sustains.
        for et in range(0, NE, 2):
            nc.sync.dma_start(out=wt[:, et : et + 2, :], in_=wt_r[:, et : et + 2, :])
            nc.sync.dma_start(
                out=xt[:, et : et + 2, 0:512], in_=xq_r[:, et : et + 2, 0:512]
            )
        for et in range(0, NE, 2):
            nc.sync.dma_start(
                out=xt[:, et : et + 2, 512:1024], in_=xq_r[:, et : et + 2, 512:1024]
            )
        nc.sync.dma_start(out=xt[:, :, 1024:2048], in_=xq_r[:, :, 1024:2048])
        for c in range(4):
            nc.sync.dma_start(
                out=xt[:, :, HALF + c * 512 : HALF + (c + 1) * 512],
                in_=xo_r[:, :, c * 512 : (c + 1) * 512],
            )

        nc.vector.memset(vt[64:65, :], 1.0)

        # one packed projection half-chunk of 512 cols
        def proj_half(w0, wm, dst, src_x0, d0):
            acc = psA.tile([128, 512], FP32, tag="ps")
            for et in range(NE):
                nc.tensor.matmul(
                    out=acc[0:wm, 0:512],
                    lhsT=wt[:, et, w0 : w0 + wm],
                    rhs=xt[:, et, src_x0 : src_x0 + 512],
                    start=(et == 0),
                    stop=(et == NE - 1),
                )
            nc.vector.tensor_copy(out=dst[:, d0 : d0 + 512], in_=acc[0:wm, 0:512])

        def shift(dst, src, d0):
            # gpsimd's SWDGE queue is otherwise idle (inputs all ride the
            # sync queue now), so shifts never wait behind bulk input
            # pieces and don't steal ScalarE time from the exps.
            nc.gpsimd.dma_start(
                out=dst[:, d0 : d0 + 512], in_=src[64:128, d0 : d0 + 512]
            )

        def shift_odd(dst, src):
            # pack the odd 128-col K^T tiles of `src` rows 0:64 into
            # `dst` rows 64:128 (base partition 64) for pass-1 row tiling
            nc.gpsimd.dma_start(
                out=dst[64:128, :].rearrange("p (t c) -> p t c", c=128),
                in_=src[0:64, :].rearrange("p (t c) -> p t c", c=128)[:, 1::2, :],
            )

        def v_transpose(k):
            tp = psA.tile([128, 512], BF16, tag="ps")
            if k < 16:  # own half: vt carries the ones row
                nc.tensor.transpose(
                    out=tp[0:128, 0:65],
                    in_=vt[:, k * 128 : (k + 1) * 128],
                    identity=identB[0:65, 0:65],
                )
                nc.vector.tensor_copy(out=vn[:, k, :], in_=tp[0:128, 0:65])
            else:  # other half: V^T shifted into vto (base partition 0)
                j = k - 16
                nc.tensor.transpose(
                    out=tp[0:128, 0:64],
                    in_=vto[:, j * 128 : (j + 1) * 128],
                    identity=identB[0:64, 0:64],
                )
                nc.vector.memset(vn[:, k, D : D + 1], 1.0)
                nc.vector.tensor_copy(out=vn[:, k, 0:D], in_=tp[0:128, 0:64])

        # fused A/C(/Q) projection lump for one 512-col chunk of own
        # q-rows. A gives [K^T;Q^T] packed; Q (prologue chunks only)
        # produces the base-0 Q^T copy directly, replacing a shift-DMA
        # whose ~5.5us latency (contending with the input stream for DMA
        # engines) sat on the scores-iter-0 critical path. (A col-tiled
        # K64 projection for free K^T@64 was tried: the framework pairs
        # every matmul with its own LDWEIGHTS, which cannot pull ahead
        # past an in-flight matmul on overlapping PE rows, so the
        # "concurrent" col-tiles serialize and cost full price.)
        def ack_lump(hh, with_q):
            x0 = hh * 512
            accA = psA.tile([128, 512], FP32, tag="ps")
            accC = psA.tile([128, 512], FP32, tag="ps")
            if with_q:
                accQ = psA.tile([128, 512], FP32, tag="ps")
            for et in range(NE):
                fl = dict(start=(et == 0), stop=(et == NE - 1))
                rhs = xt[:, et, x0 : x0 + 512]
                nc.tensor.matmul(
                    out=accA[0:128, 0:512], lhsT=wt[:, et, 0:128], rhs=rhs, **fl
                )
                nc.tensor.matmul(
                    out=accC[0:64, 0:512], lhsT=wt[:, et, 256:320], rhs=rhs, **fl
                )
                if with_q:
                    nc.tensor.matmul(
                        out=accQ[0:64, 0:512], lhsT=wt[:, et, 320:384], rhs=rhs, **fl
                    )
            if with_q:
                # qts gates scores iter 0: copy it first
                nc.vector.tensor_copy(
                    out=qts[:, x0 : x0 + 512], in_=accQ[0:64, 0:512]
                )
            nc.vector.tensor_copy(out=qk[:, x0 : x0 + 512], in_=accA[0:128, 0:512])
            nc.vector.tensor_copy(out=vt[0:64, x0 : x0 + 512], in_=accC[0:64, 0:512])

        def exp_half(sc, p, c, eng):
            # eng 'S': exact ACT exp on ScalarE. 'V': one-instruction
            # Schraudolph bit-trick on VectorE (see module docstring).
            dst = p[:, c * 512 : (c + 1) * 512]
            if eng == "S":
                nc.scalar.activation(out=dst, in_=sc[:, :], func=Exp, scale=SCALE)
            else:
                nc.vector.tensor_scalar(
                    dst.bitcast(I16), sc[:, :], A16, B16, Mult, Add
                )

        # --- prologue: chunks 0-1 (own q-cols 0:1024), consumed et-by-et
        # as the pieces land. Between the chunks, the c0-half scores of
        # k-tiles 0-3 (all inputs come from chunk 0) fill the DMA bubbles
        # of the arrival-paced chunk-1 projections.
        ack_lump(0, with_q=True)
        early_p = {}
        for k in range(4):
            sc = psA.tile([128, 512], FP32, tag="ps")
            nc.tensor.matmul(
                out=sc[:, :], lhsT=qk[0:64, k * 128 : (k + 1) * 128],
                rhs=qts[:, 0:512], start=True, stop=True,
            )
            p = pp.tile([128, 1024], BF16)
            exp_half(sc, p, 0, "SV"[k % 2])
            early_p[k] = p
        ack_lump(1, with_q=True)

        # pass-0 side-slot schedule: iter k -> own-chunk ACK lump or oth
        # chunk B projection. Slots track the ordered-queue arrivals (own
        # 1024:2048 by ~24us, oth chunk c by ~27+3c); deadlines: ACK hh
        # feeds scores k=4hh, B-lump hh feeds scores k=16+4hh. The ko/kq
        # odd-tile shifts and qts shifts only feed pass 1 (loose), and
        # the attn lag-5 backlog bridges the arrival waits.
        SIDE = {4: ("A", 2), 5: ("C", 2), 6: ("A", 3), 7: ("C", 3),
                9: ("B", 0), 11: ("B", 1), 13: ("B", 2), 15: ("B", 3)}

        def side_work(k):
            s = SIDE.get(k)
            if s is not None:
                kind, hh = s
                if kind == "A":
                    proj_half(0, 128, qk, hh * 512, hh * 512)
                    shift(qts, qk, hh * 512)
                    if hh == 3:
                        shift_odd(ko, qk)
                elif kind == "C":
                    proj_half(256, 64, vt[0:64, :], hh * 512, hh * 512)
                else:
                    proj_half(128, 128, kv, HALF + hh * 512, hh * 512)
                    shift(vto, kv, hh * 512)
                    if hh == 3:
                        shift_odd(kq, kv)
            if k == 2:
                v_transpose(0)
                v_transpose(1)
            elif k >= 3:
                v_transpose(k - 1)
                if k == NKT - 1:
                    v_transpose(NKT - 1)

        out_engs = [nc.sync, nc.gpsimd]

        def ship(ps, att_ps):
            # ship this pass's attn^T + denominators in bf16 (host
            # normalizes in fp32) as four 256-col chunks: copies
            # alternate VectorE/ScalarE and the DMAs alternate the
            # sync/gpsimd queues, so four short copy+DMA chains overlap
            # in the tail instead of two long ones.
            for c in range(4):
                cols = slice(ps * 1024 + c * 256, ps * 1024 + (c + 1) * 256)
                pcols = slice(c * 256, (c + 1) * 256)
                if c % 2 == 0:
                    nc.vector.tensor_copy(
                        out=att_sb[:, cols], in_=att_ps[0:65, pcols]
                    )
                else:
                    nc.scalar.activation(
                        out=att_sb[:, cols], in_=att_ps[0:65, pcols], func=Cpy
                    )
                out_engs[c % 2].dma_start(out=out_d[:, cols], in_=att_sb[:, cols])

        # row-tiled score pair for k-tiles (2j, 2j+1): the even tile
        # (K^T at base partition 0, Q^T copy at base 0) and the odd tile
        # (K^T at base 64, Q^T at base 64 straight out of the packed
        # A-projection) occupy disjoint PE row-groups, so their matmuls
        # run concurrently (measured dstart ~4ns): scores cost halves.
        def score_pair(j, q_base, klhs_e, klhs_o):
            p_e = pp.tile([128, 1024], BF16)
            p_o = pp.tile([128, 1024], BF16)
            for c in range(2):
                q0 = q_base + c * 512
                sc_e = psA.tile([128, 512], FP32, tag="ps")
                sc_o = psA.tile([128, 512], FP32, tag="ps")
                nc.tensor.matmul(
                    out=sc_e[:, :], lhsT=klhs_e, rhs=qts[:, q0 : q0 + 512],
                    start=True, stop=True,
                )
                nc.tensor.matmul(
                    out=sc_o[:, :], lhsT=klhs_o, rhs=qk[64:128, q0 : q0 + 512],
                    start=True, stop=True,
                )
                exp_half(sc_e, p_e, c, "SV"[c])
                exp_half(sc_o, p_o, c, "VS"[c])
            p_tiles[2 * j] = p_e
            p_tiles[2 * j + 1] = p_o

        # --- pass 0: q-cols 0:1024, plain scores + side work. The attn
        # lag-5 backlog doubles as a work reservoir bridging input-piece
        # arrival waits; the balanced per-half S/V exp split keeps either
        # engine under the iteration wall.
        att_ps = psB.tile([128, 1024], FP32)
        p_tiles = {}
        nxt = [0]

        def drain(upto):
            while nxt[0] <= upto:
                _attn(nc, att_ps, vn, p_tiles, nxt[0])
                nxt[0] += 1

        for k in range(NKT):
            if k < 16:
                klhs = qk[0:64, k * 128 : (k + 1) * 128]
            else:
                klhs = kv[0:64, (k - 16) * 128 : (k - 15) * 128]
            if k < 4:
                p = early_p.pop(k)
                halves = (1,)
            else:
                p = pp.tile([128, 1024], BF16)
                halves = (0, 1)
            for c in halves:
                sc = psA.tile([128, 512], FP32, tag="ps")
                nc.tensor.matmul(
                    out=sc[:, :], lhsT=klhs, rhs=qts[:, c * 512 : (c + 1) * 512],
                    start=True, stop=True,
                )
                exp_half(sc, p, c, "SV"[(k + c) % 2])
            p_tiles[k] = p
            side_work(k)
            drain(k - 5)
        drain(NKT - 1)
        ship(0, att_ps)

        # --- pass 1: q-cols 1024:2048, all 16 pairs row-tiled (odd
        # tiles' K^T from the ko/kq shifts done during pass 0)
        att_ps = psB.tile([128, 1024], FP32)
        nxt = [0]
        for j in range(NKT // 2):
            ke = 2 * j
            if j < 8:
                klhs_e = qk[0:64, ke * 128 : (ke + 1) * 128]
                klhs_o = ko[64:128, j * 128 : (j + 1) * 128]
            else:
                klhs_e = kv[0:64, (ke - 16) * 128 : (ke - 15) * 128]
                klhs_o = kq[64:128, (j - 8) * 128 : (j - 7) * 128]
            score_pair(j, 1024, klhs_e, klhs_o)
            drain(ke + 1 - 5)
        drain(NKT - 1)
        ship(1, att_ps)

    nc.compile()
    _CACHE["nc"] = nc
    return nc


def _attn(nc, att_ps, vn, p_tiles, k):
    p = p_tiles.pop(k)
    for c in range(2):
        nc.tensor.matmul(
            out=att_ps[0:65, c * 512 : (c + 1) * 512],
            lhsT=vn[:, k, :],
            rhs=p[:, c * 512 : (c + 1) * 512],
            start=(k == 0),
            stop=(k == NKT - 1),
            skip_group_check=True,
        )


def _make_in_maps(x, Wq, Wk, Wv):
    import ml_dtypes

    bf16 = ml_dtypes.bfloat16
    xT = np.ascontiguousarray(x.transpose(0, 2, 1)).astype(bf16)  # [B, E, S]
    wt = np.concatenate(
        [Wk.T, Wq.T, Wk.T, Wv.T, Wv.T, Wq.T], axis=1
    ).astype(bf16)  # [E, 384]
    in_maps = []
    for c in range(N_CORES):
        b, h = divmod(c, 2)
        in_maps.append(
            {
                "xt_q": np.ascontiguousarray(xT[b, :, h * HALF : (h + 1) * HALF]),
                "xt_o": np.ascontiguousarray(
                    xT[b, :, (1 - h) * HALF : (2 - h) * HALF]
                ),
                "wt": wt,
            }
        )
    return in_maps


def _run(x, Wq, Wk, Wv, trace=False):
    from concourse.bass_utils import run_bass_kernel_spmd

    nc = _build()
    in_maps = _make_in_maps(x, Wq, Wk, Wv)
    res = run_bass_kernel_spmd(
        nc, in_maps, core_ids=list(range(N_CORES)), trace=trace
    )
    out = np.empty((B, S, D), dtype=np.float32)
    for c in range(N_CORES):
        b, h = divmod(c, 2)
        att = np.asarray(
            res.results[c]["out"], dtype=np.float32
        )  # [65, HALF]: attn^T rows + denom row (bf16 on the wire)
        out[b, h * HALF : (h + 1) * HALF, :] = (att[0:D] / att[D : D + 1]).T
    return out, res


def kernel(x, Wq, Wk, Wv):
    out, _ = _run(
        np.asarray(x, dtype=np.float32),
        np.asarray(Wq, dtype=np.float32),
        np.asarray(Wk, dtype=np.float32),
        np.asarray(Wv, dtype=np.float32),
    )
    return out



# revision 46
# speedup vs baseline: 1.0027x; 1.0027x over previous
"""Single-head attention (B=4, S=4096, E=1024, D=64) on 8 TRN2 NeuronCores.

Sharding: data-parallel over (batch, query-half): core c handles batch
b = c // 2 and query rows [h*2048, (h+1)*2048) with h = c % 2. Each core
computes Q for its own 2048 rows and K/V for the full 4096 rows of its batch
(inputs are shipped host-pretransposed per half, so no duplicated DMA).

Per-core dataflow (TensorE matmuls in bf16 — fp32/fp32r matmuls run the PE
at half clock; fp32 accumulation in PSUM). Projections pack TWO weight
matrices into one 128-wide stationary operand:
  qk [128, 2048] = [K^T_own; Q^T_own]     (pass A, lhsT = [WkT | WqT])
  kv [128, 2048] = [K^T_oth; V^T_oth]     (pass B, lhsT = [WkT | WvT])
  vt [65, 2048]  = V^T_own + ones row      (pass C, lhsT = WvT)
Q^T / V^T_oth are then shifted to base partition 0 by SBUF-to-SBUF DMAs
(matmul operands must share a base partition).
  scores^T[k, q] = K^T.T @ Q^T -> exp -> P bf16
  attn^T[65, q] += V_aug.T @ P   (row 64 accumulates softmax denominators)
  output = attn^T with denominators; host transposes + normalizes.

The exp is split across two engines so ScalarE (1 elem/cycle/lane at
1.2 GHz, ~1.15us per [128,1024] tile) stops pacing the pipeline: 2/3 of
k-tiles get the exact ACT exp on ScalarE; every third tile is computed on
VectorE with a one-instruction Schraudolph bit-trick: i16 = rne(x*A + B)
reinterpreted as bf16 approximates exp(SCALE*x) (piecewise-linear mantissa
chord, ~1.8% log-error sawtooth whose mean bias cancels in the softmax
numerator/denominator ratio; applied to 20/64 of the weights it adds
~0.6% output rel err). The two engines use SEPARATE P-tile pools — a
shared pool serializes them via buffer-reuse ordering.

The attention runs as TWO passes over q (1024 columns each): the attn
accumulator then fits 2 PSUM banks, freeing a third scores slot (PSUM slot
contention paced the single-pass version), and pass 0's output ships
mid-kernel.

The HAM duty controller halves the PE clock after ~2.5us of PE idleness
and takes 5-13us at half clock to re-grant full duty, so the kernel keeps
the PE streaming: junk-fed 512-col warm-up matmuls run from the instant
the PE preamble ends until the first input DMAs land (gated only on a
VectorE memset, not on make_identity's gpsimd iota), and junk fillers
bridge the group-2 DMA wait at pass-0 iters 4-5. Input DMA is issued in
deadline order (wt + own q-cols 0:1024, then own 1024:2048, then the
other half) across the sync/scalar/gpsimd queues; the pass-0 side-slot
schedule (projection lumps A2/C2/A3/C3, then B0-B3) tracks the measured
arrival of those groups.
"""

import numpy as np

B, S, E, D = 4, 4096, 1024, 64
HALF = S // 2
N_CORES = 8
SCALE = 1.0 / np.sqrt(D)

NE = E // 128  # 8 e-tiles
NKT = S // 128  # 32 k-tiles
N_WARM = 11  # 512-col PE warm-up matmuls covering the preamble + DMA wait

# Schraudolph exp-approx constants (bf16 bit pattern via int16):
#   i16 = round(x * A16 + B16); bitcast bf16 ~= exp(SCALE * x)
LOG2E = 1.4426950408889634
A16 = SCALE * 128.0 * LOG2E
B16 = 127.0 * 128.0 - 7.3


_CACHE = {}


def _build():
    if "nc" in _CACHE:
        return _CACHE["nc"]

    from contextlib import ExitStack

    import concourse.bacc as bacc
    import concourse.tile as tile
    from concourse import mybir
    from concourse.masks import make_identity

    FP32 = mybir.dt.float32
    BF16 = mybir.dt.bfloat16
    I16 = mybir.dt.int16
    Exp = mybir.ActivationFunctionType.Exp
    Cpy = mybir.ActivationFunctionType.Copy
    Mult = mybir.AluOpType.mult
    Add = mybir.AluOpType.add

    nc = bacc.Bacc(
        "TRN2", target_bir_lowering=False, debug=False, num_devices=N_CORES
    )

    xt_q_d = nc.dram_tensor("xt_q", [E, HALF], BF16, kind="ExternalInput").ap()
    xt_o_d = nc.dram_tensor("xt_o", [E, HALF], BF16, kind="ExternalInput").ap()
    wt_d = nc.dram_tensor("wt", [E, 384], BF16, kind="ExternalInput").ap()
    out_d = nc.dram_tensor("out", [D + 1, HALF], BF16, kind="ExternalOutput").ap()

    with tile.TileContext(nc) as tc, ExitStack() as ctx:
        const = ctx.enter_context(tc.tile_pool(name="const", bufs=1))
        big = ctx.enter_context(tc.tile_pool(name="big", bufs=1))
        # P tiles: each [128,1024] tile gets its c0/c1 halves written by
        # DIFFERENT exp engines (balanced S/V split), so one pool is fine —
        # slot reuse gates both engines on the same attn read.
        pp = ctx.enter_context(tc.tile_pool(name="pp", bufs=7))
        # psA: six single-bank [128,512] score/side slots; psB: the 2-bank
        # attn accumulator. 6*2KB + 4KB = 16KB = all 8 PSUM banks.
        psA = ctx.enter_context(tc.tile_pool(name="psA", bufs=6, space="PSUM"))
        psB = ctx.enter_context(tc.tile_pool(name="psB", bufs=1, space="PSUM"))

        identB = const.tile([128, 128], BF16)
        junk = const.tile([128, 512], BF16)
        # memset on gpsimd: its preamble ends ~1us before vector's, so
        # the PE warm-up starts that much earlier.
        nc.gpsimd.memset(junk[:, :], 0.0)
        make_identity(nc, identB)

        xt = big.tile([128, NE, S], BF16)  # x^T; cols [0, HALF) = own q-rows
        wt = big.tile([128, NE, 384], BF16)  # [WkT|WqT | WkT|WvT | WvT | WqT]
        qk = big.tile([128, HALF], BF16)  # rows 0-63 K^T own, 64-127 Q^T own
        kv = big.tile([128, HALF], BF16)  # rows 0-63 K^T oth, 64-127 V^T oth
        qts = big.tile([64, HALF], BF16)  # Q^T shifted to base partition 0
        vto = big.tile([64, HALF], BF16)  # V^T other shifted to base part. 0
        vt = big.tile([65, HALF], BF16)  # V^T own; row 64 = ones
        vn = big.tile([128, NKT, D + 1], BF16)  # V natural + ones column
        att_sb = big.tile([65, HALF], BF16)  # attn^T + denominator row
        # odd K^T tiles shifted to base partition 64 for pass-1 row
        # tiling (rows 64-127: own tiles 1,3..15 / oth tiles 17,19..31)
        ko = big.tile([128, 1024], BF16)
        kq = big.tile([128, 1024], BF16)

        # --- PE warm-up: the HAM duty controller halves the PE clock after
        # ~2.5us of idleness and takes 5-13us at half clock to restore full
        # duty, so keep the PE streaming junk matmuls from the instant its
        # preamble ends until the first input DMAs land.
        warm = psA.tile([128, 512], FP32, tag="ps")
        for _ in range(N_WARM):
            nc.tensor.matmul(
                out=warm[0:128, 0:512],
                lhsT=junk[:, 0:128],
                rhs=junk[:, :],
                start=True,
                stop=True,
            )

        # --- input DMAs: ALL on the sync queue, issued in consumption
        # order. A single queue drains FIFO across all 16 DMA engines, so
        # arrival order == issue order and the PE never waits on a piece
        # that lost a queue-arbitration race (the old 3-queue split left
        # 12.6us of mid-ramp PE idle + HAM re-throttles). Measured stream
        # rate ~0.32 MB/us: wt ~9.5us, own 0:512 ~12.8, 512:1024 ~16.1,
        # own 1024:2048 ~22.7, oth chunks ~26/29/33/36us.
        xq_r = xt_q_d.rearrange("(t p) s -> p t s", p=128)
        xo_r = xt_o_d.rearrange("(t p) s -> p t s", p=128)
        wt_r = wt_d.rearrange("(t p) d -> p t d", p=128)
        # wt and own cols 0:512 interleaved per e-tile PAIR so the fused
        # A/C/Q prologue's et-pair k needs only the first 2k+2 pieces —
        # first useful matmul ~3us earlier than with a monolithic wt load.
        # Pieces stay >=192KB: below ~256KB the ~625ns-per-start HWDGE
        # config rate on sync throttles the stream under the ~0.36MB/us
        # the HBM path sustains.
        for et in range(0, NE, 2):
            nc.sync.dma_start(out=wt[:, et : et + 2, :], in_=wt_r[:, et : et + 2, :])
            nc.sync.dma_start(
                out=xt[:, et : et + 2, 0:512], in_=xq_r[:, et : et + 2, 0:512]
            )
        for et in range(0, NE, 2):
            nc.sync.dma_start(
                out=xt[:, et : et + 2, 512:1024], in_=xq_r[:, et : et + 2, 512:1024]
            )
        nc.sync.dma_start(out=xt[:, :, 1024:2048], in_=xq_r[:, :, 1024:2048])
        for c in range(4):
            nc.sync.dma_start(
                out=xt[:, :, HALF + c * 512 : HALF + (c + 1) * 512],
                in_=xo_r[:, :, c * 512 : (c + 1) * 512],
            )

        nc.vector.memset(vt[64:65, :], 1.0)

        # one packed projection half-chunk of 512 cols
        def proj_half(w0, wm, dst, src_x0, d0):
            acc = psA.tile([128, 512], FP32, tag="ps")
            for et in range(NE):
                nc.tensor.matmul(
                    out=acc[0:wm, 0:512],
                    lhsT=wt[:, et, w0 : w0 + wm],
                    rhs=xt[:, et, src_x0 : src_x0 + 512],
                    start=(et == 0),
                    stop=(et == NE - 1),
                )
            nc.vector.tensor_copy(out=dst[:, d0 : d0 + 512], in_=acc[0:wm, 0:512])

        def shift(dst, src, d0):
            # gpsimd's SWDGE queue is otherwise idle (inputs all ride the
            # sync queue now), so shifts never wait behind bulk input
            # pieces and don't steal ScalarE time from the exps.
            nc.gpsimd.dma_start(
                out=dst[:, d0 : d0 + 512], in_=src[64:128, d0 : d0 + 512]
            )

        def shift_odd(dst, src):
            # pack the odd 128-col K^T tiles of `src` rows 0:64 into
            # `dst` rows 64:128 (base partition 64) for pass-1 row tiling
            nc.gpsimd.dma_start(
                out=dst[64:128, :].rearrange("p (t c) -> p t c", c=128),
                in_=src[0:64, :].rearrange("p (t c) -> p t c", c=128)[:, 1::2, :],
            )

        def v_transpose(k):
            tp = psA.tile([128, 512], BF16, tag="ps")
            if k < 16:  # own half: vt carries the ones row
                nc.tensor.transpose(
                    out=tp[0:128, 0:65],
                    in_=vt[:, k * 128 : (k + 1) * 128],
                    identity=identB[0:65, 0:65],
                )
                nc.vector.tensor_copy(out=vn[:, k, :], in_=tp[0:128, 0:65])
            else:  # other half: V^T shifted into vto (base partition 0)
                j = k - 16
                nc.tensor.transpose(
                    out=tp[0:128, 0:64],
                    in_=vto[:, j * 128 : (j + 1) * 128],
                    identity=identB[0:64, 0:64],
                )
                nc.vector.memset(vn[:, k, D : D + 1], 1.0)
                nc.vector.tensor_copy(out=vn[:, k, 0:D], in_=tp[0:128, 0:64])

        # fused A/C(/Q) projection lump for one 512-col chunk of own
        # q-rows. A gives [K^T;Q^T] packed; Q (prologue chunks only)
        # produces the base-0 Q^T copy directly, replacing a shift-DMA
        # whose ~5.5us latency (contending with the input stream for DMA
        # engines) sat on the scores-iter-0 critical path. (A col-tiled
        # K64 projection for free K^T@64 was tried: the framework pairs
        # every matmul with its own LDWEIGHTS, which cannot pull ahead
        # past an in-flight matmul on overlapping PE rows, so the
        # "concurrent" col-tiles serialize and cost full price.)
        def ack_lump(hh, with_q):
            x0 = hh * 512
            accA = psA.tile([128, 512], FP32, tag="ps")
            accC = psA.tile([128, 512], FP32, tag="ps")
            if with_q:
                accQ = psA.tile([128, 512], FP32, tag="ps")
            for et in range(NE):
                fl = dict(start=(et == 0), stop=(et == NE - 1))
                rhs = xt[:, et, x0 : x0 + 512]
                nc.tensor.matmul(
                    out=accA[0:128, 0:512], lhsT=wt[:, et, 0:128], rhs=rhs, **fl
                )
                nc.tensor.matmul(
                    out=accC[0:64, 0:512], lhsT=wt[:, et, 256:320], rhs=rhs, **fl
                )
                if with_q:
                    nc.tensor.matmul(
                        out=accQ[0:64, 0:512], lhsT=wt[:, et, 320:384], rhs=rhs, **fl
                    )
            if with_q:
                # qts gates scores iter 0: copy it first
                nc.vector.tensor_copy(
                    out=qts[:, x0 : x0 + 512], in_=accQ[0:64, 0:512]
                )
            nc.vector.tensor_copy(out=qk[:, x0 : x0 + 512], in_=accA[0:128, 0:512])
            nc.vector.tensor_copy(out=vt[0:64, x0 : x0 + 512], in_=accC[0:64, 0:512])

        # --- prologue: chunks 0-1 (own q-cols 0:1024), consumed et-by-et
        # as the pieces land ---
        for hh in range(2):
            ack_lump(hh, with_q=True)

        # pass-0 side-slot schedule: iter k -> own-chunk ACK lump or oth
        # chunk B projection. Slots track the ordered-queue arrivals (own
        # 1024:2048 by ~24us, oth chunk c by ~27+3c); deadlines: ACK hh
        # feeds scores k=4hh, B-lump hh feeds scores k=16+4hh. The ko/kq
        # odd-tile shifts and qts shifts only feed pass 1 (loose), and
        # the attn lag-5 backlog bridges the arrival waits.
        SIDE = {4: ("A", 2), 5: ("C", 2), 6: ("A", 3), 7: ("C", 3),
                9: ("B", 0), 11: ("B", 1), 13: ("B", 2), 15: ("B", 3)}

        def side_work(k):
            s = SIDE.get(k)
            if s is not None:
                kind, hh = s
                if kind == "A":
                    proj_half(0, 128, qk, hh * 512, hh * 512)
                    shift(qts, qk, hh * 512)
                    if hh == 3:
                        shift_odd(ko, qk)
                elif kind == "C":
                    proj_half(256, 64, vt[0:64, :], hh * 512, hh * 512)
                else:
                    proj_half(128, 128, kv, HALF + hh * 512, hh * 512)
                    shift(vto, kv, hh * 512)
                    if hh == 3:
                        shift_odd(kq, kv)
            if k == 2:
                v_transpose(0)
                v_transpose(1)
            elif k >= 3:
                v_transpose(k - 1)
                if k == NKT - 1:
                    v_transpose(NKT - 1)

        out_engs = [nc.sync, nc.scalar]

        def exp_half(sc, p, c, eng):
            # eng 'S': exact ACT exp on ScalarE. 'V': one-instruction
            # Schraudolph bit-trick on VectorE (see module docstring).
            dst = p[:, c * 512 : (c + 1) * 512]
            if eng == "S":
                nc.scalar.activation(out=dst, in_=sc[:, :], func=Exp, scale=SCALE)
            else:
                nc.vector.tensor_scalar(
                    dst.bitcast(I16), sc[:, :], A16, B16, Mult, Add
                )

        def ship(ps, att_ps):
            # ship this pass's attn^T + denominators in bf16 (host
            # normalizes in fp32); the c0/c1 copies ride different
            # engines so the tail's two copy+DMA chains overlap.
            for c in range(2):
                cols = slice(ps * 1024 + c * 512, ps * 1024 + (c + 1) * 512)
                pcols = slice(c * 512, (c + 1) * 512)
                if c == 0:
                    nc.vector.tensor_copy(
                        out=att_sb[:, cols], in_=att_ps[0:65, pcols]
                    )
                else:
                    nc.scalar.activation(
                        out=att_sb[:, cols], in_=att_ps[0:65, pcols], func=Cpy
                    )
                out_engs[c].dma_start(out=out_d[:, cols], in_=att_sb[:, cols])

        # row-tiled score pair for k-tiles (2j, 2j+1): the even tile
        # (K^T at base partition 0, Q^T copy at base 0) and the odd tile
        # (K^T at base 64, Q^T at base 64 straight out of the packed
        # A-projection) occupy disjoint PE row-groups, so their matmuls
        # run concurrently (measured dstart ~4ns): scores cost halves.
        def score_pair(j, q_base, klhs_e, klhs_o):
            p_e = pp.tile([128, 1024], BF16)
            p_o = pp.tile([128, 1024], BF16)
            for c in range(2):
                q0 = q_base + c * 512
                sc_e = psA.tile([128, 512], FP32, tag="ps")
                sc_o = psA.tile([128, 512], FP32, tag="ps")
                nc.tensor.matmul(
                    out=sc_e[:, :], lhsT=klhs_e, rhs=qts[:, q0 : q0 + 512],
                    start=True, stop=True,
                )
                nc.tensor.matmul(
                    out=sc_o[:, :], lhsT=klhs_o, rhs=qk[64:128, q0 : q0 + 512],
                    start=True, stop=True,
                )
                exp_half(sc_e, p_e, c, "SV"[c])
                exp_half(sc_o, p_o, c, "VS"[c])
            p_tiles[2 * j] = p_e
            p_tiles[2 * j + 1] = p_o

        # --- pass 0: q-cols 0:1024, plain scores + side work. The attn
        # lag-5 backlog doubles as a work reservoir bridging input-piece
        # arrival waits; the balanced per-half S/V exp split keeps either
        # engine under the iteration wall.
        att_ps = psB.tile([128, 1024], FP32)
        p_tiles = {}
        nxt = [0]

        def drain(upto):
            while nxt[0] <= upto:
                _attn(nc, att_ps, vn, p_tiles, nxt[0])
                nxt[0] += 1

        for k in range(NKT):
            if k < 16:
                klhs = qk[0:64, k * 128 : (k + 1) * 128]
            else:
                klhs = kv[0:64, (k - 16) * 128 : (k - 15) * 128]
            p = pp.tile([128, 1024], BF16)
            for c in range(2):
                sc = psA.tile([128, 512], FP32, tag="ps")
                nc.tensor.matmul(
                    out=sc[:, :], lhsT=klhs, rhs=qts[:, c * 512 : (c + 1) * 512],
                    start=True, stop=True,
                )
                exp_half(sc, p, c, "SV"[(k + c) % 2])
            p_tiles[k] = p
            side_work(k)
            drain(k - 5)
        drain(NKT - 1)
        ship(0, att_ps)

        # --- pass 1: q-cols 1024:2048, all 16 pairs row-tiled (odd
        # tiles' K^T from the ko/kq shifts done during pass 0)
        att_ps = psB.tile([128, 1024], FP32)
        nxt = [0]
        for j in range(NKT // 2):
            ke = 2 * j
            if j < 8:
                klhs_e = qk[0:64, ke * 128 : (ke + 1) * 128]
                klhs_o = ko[64:128, j * 128 : (j + 1) * 128]
            else:
                klhs_e = kv[0:64, (ke - 16) * 128 : (ke - 15) * 128]
                klhs_o = kq[64:128, (j - 8) * 128 : (j - 7) * 128]
            score_pair(j, 1024, klhs_e, klhs_o)
            drain(ke + 1 - 5)
        drain(NKT - 1)
        ship(1, att_ps)

    nc.compile()
    _CACHE["nc"] = nc
    return nc


def _attn(nc, att_ps, vn, p_tiles, k):
    p = p_tiles.pop(k)
    for c in range(2):
        nc.tensor.matmul(
            out=att_ps[0:65, c * 512 : (c + 1) * 512],
            lhsT=vn[:, k, :],
            rhs=p[:, c * 512 : (c + 1) * 512],
            start=(k == 0),
            stop=(k == NKT - 1),
            skip_group_check=True,
        )


def _make_in_maps(x, Wq, Wk, Wv):
    import ml_dtypes

    bf16 = ml_dtypes.bfloat16
    xT = np.ascontiguousarray(x.transpose(0, 2, 1)).astype(bf16)  # [B, E, S]
    wt = np.concatenate(
        [Wk.T, Wq.T, Wk.T, Wv.T, Wv.T, Wq.T], axis=1
    ).astype(bf16)  # [E, 384]
    in_maps = []
    for c in range(N_CORES):
        b, h = divmod(c, 2)
        in_maps.append(
            {
                "xt_q": np.ascontiguousarray(xT[b, :, h * HALF : (h + 1) * HALF]),
                "xt_o": np.ascontiguousarray(
                    xT[b, :, (1 - h) * HALF : (2 - h) * HALF]
                ),
                "wt": wt,
            }
        )
    return in_maps


def _run(x, Wq, Wk, Wv, trace=False):
    from concourse.bass_utils import run_bass_kernel_spmd

    nc = _build()
    in_maps = _make_in_maps(x, Wq, Wk, Wv)
    res = run_bass_kernel_spmd(
        nc, in_maps, core_ids=list(range(N_CORES)), trace=trace
    )
    out = np.empty((B, S, D), dtype=np.float32)
    for c in range(N_CORES):
        b, h = divmod(c, 2)
        att = np.asarray(
            res.results[c]["out"], dtype=np.float32
        )  # [65, HALF]: attn^T rows + denom row (bf16 on the wire)
        out[b, h * HALF : (h + 1) * HALF, :] = (att[0:D] / att[D : D + 1]).T
    return out, res


def kernel(x, Wq, Wk, Wv):
    out, _ = _run(
        np.asarray(x, dtype=np.float32),
        np.asarray(Wq, dtype=np.float32),
        np.asarray(Wk, dtype=np.float32),
        np.asarray(Wv, dtype=np.float32),
    )
    return out



# revision 47
# speedup vs baseline: 1.2140x; 1.2106x over previous
"""Single-head attention (B=4, S=4096, E=1024, D=64) on 8 TRN2 NeuronCores.

Sharding: data-parallel over (batch, query-half): core c handles batch
b = c // 2 and query rows [h*2048, (h+1)*2048) with h = c % 2. Each core
computes Q for its own 2048 rows and K/V for the full 4096 rows of its
batch (inputs are shipped host-pretransposed per half).

Per-core dataflow (TensorE matmuls in bf16; fp32 accumulation in PSUM).
Projections pack weight matrices into wide stationary operands:
  qk [128, 2048] = [K^T_own; Q^T_own]   (lump A, lhsT = [WkT | WqT])
  kv [128, 2048] = [K^T_oth; V^T_oth]   (lump B, lhsT = [WkT | WvT])
  vt [65, 2048]  = V^T_own + ones row   (lump C, lhsT = WvT)
  qts [64, 2048] = Q^T at base partition 0 (prologue lump Q, lhsT = WqT;
      a direct projection because the SBUF-SBUF shift-DMA alternative has
      ~5.5us latency under input-stream contention and sat on the
      scores-iter-0 critical path; pass-1 columns DO use shift-DMAs since
      their deadline is a whole pass away)
  scores^T[k, q] = K^T.T @ Q^T -> exp -> P bf16
  attn^T[65, q] += V_aug.T @ P   (row 64 accumulates softmax denominators)
  output = attn^T + denominators in bf16; host transposes + normalizes.

Input DMA rides ONE queue (sync HWDGE) in strict consumption order --
wt/x interleaved per e-tile pair, then own cols 512:1024, own 1024:2048,
then the other half in 512-col chunks. A single queue drains FIFO across
all 16 DMA engines at ~0.36MB/us, so arrival order == issue order and the
PE never waits on a piece that lost a queue-arbitration race. Pieces stay
>=192KB so the ~625ns-per-start config rate keeps ahead of the stream.
The fused A/C/Q prologue consumes pieces et-by-et as they land; the
pass-0 side-slot schedule (A2/C2/A3/C3 then B0-B3) tracks arrivals, and
the attn lag-5 backlog bridges residual arrival jitter.

The scores matmul contracts over only D=64 of the PE's 128 rows, so
pass 1 runs k-tiles as ROW-TILED PAIRS: the even tile (K^T at base
partition 0, streaming the qts copy) and the odd tile (K^T shifted to
base 64 during pass 0, streaming Q^T straight from qk rows 64:127)
occupy disjoint PE row-groups and their matmuls run concurrently
(measured dstart ~4ns) -- scores throughput doubles. Pass 0 stays plain:
its pace is set by side-lump projections and input arrival anyway, and
the base-64 K^T copies do not exist yet.

The exp is split per 512-col half across both engines (ScalarE exact ACT
exp ~0.6us/half; VectorE one-instruction Schraudolph bit-trick:
i16 = rne(x*A + B) bitcast to bf16 ~= exp(SCALE*x), ~1.8% sawtooth whose
mean bias cancels in the softmax ratio; on half the weights it adds
~0.7% output rel err). PSUM: six single-bank [128,512] score/side slots
+ the 2-bank attn accumulator fill all 8 banks.

The HAM duty controller halves the PE clock after ~3.4us of idleness and
takes ~3.4us of sustained work to re-grant full duty, so junk warm-up
matmuls (gated on a gpsimd memset) keep the PE streaming from preamble
end until the first input pieces land; with the ordered DMA there are no
mid-kernel idle windows and the PE stays at full clock throughout.
"""

import numpy as np

B, S, E, D = 4, 4096, 1024, 64
HALF = S // 2
N_CORES = 8
SCALE = 1.0 / np.sqrt(D)

NE = E // 128  # 8 e-tiles
NKT = S // 128  # 32 k-tiles
N_WARM = 11  # 512-col PE warm-up matmuls covering the preamble + DMA wait

# Schraudolph exp-approx constants (bf16 bit pattern via int16):
#   i16 = round(x * A16 + B16); bitcast bf16 ~= exp(SCALE * x)
LOG2E = 1.4426950408889634
A16 = SCALE * 128.0 * LOG2E
B16 = 127.0 * 128.0 - 7.3


_CACHE = {}


def _build():
    if "nc" in _CACHE:
        return _CACHE["nc"]

    from contextlib import ExitStack

    import concourse.bacc as bacc
    import concourse.tile as tile
    from concourse import mybir
    from concourse.masks import make_identity

    FP32 = mybir.dt.float32
    BF16 = mybir.dt.bfloat16
    I16 = mybir.dt.int16
    Exp = mybir.ActivationFunctionType.Exp
    Cpy = mybir.ActivationFunctionType.Copy
    Mult = mybir.AluOpType.mult
    Add = mybir.AluOpType.add

    nc = bacc.Bacc(
        "TRN2", target_bir_lowering=False, debug=False, num_devices=N_CORES
    )

    xt_q_d = nc.dram_tensor("xt_q", [E, HALF], BF16, kind="ExternalInput").ap()
    xt_o_d = nc.dram_tensor("xt_o", [E, HALF], BF16, kind="ExternalInput").ap()
    wt_d = nc.dram_tensor("wt", [E, 384], BF16, kind="ExternalInput").ap()
    out_d = nc.dram_tensor("out", [D + 1, HALF], BF16, kind="ExternalOutput").ap()

    with tile.TileContext(nc) as tc, ExitStack() as ctx:
        const = ctx.enter_context(tc.tile_pool(name="const", bufs=1))
        big = ctx.enter_context(tc.tile_pool(name="big", bufs=1))
        # P tiles: each [128,1024] tile gets its c0/c1 halves written by
        # DIFFERENT exp engines (balanced S/V split), so one pool is fine —
        # slot reuse gates both engines on the same attn read.
        pp = ctx.enter_context(tc.tile_pool(name="pp", bufs=7))
        # psA: six single-bank [128,512] score/side slots; psB: the 2-bank
        # attn accumulator. 6*2KB + 4KB = 16KB = all 8 PSUM banks.
        psA = ctx.enter_context(tc.tile_pool(name="psA", bufs=6, space="PSUM"))
        psB = ctx.enter_context(tc.tile_pool(name="psB", bufs=1, space="PSUM"))

        identB = const.tile([128, 128], BF16)
        junk = const.tile([128, 512], BF16)
        # memset on gpsimd: its preamble ends ~1us before vector's, so
        # the PE warm-up starts that much earlier.
        nc.gpsimd.memset(junk[:, :], 0.0)
        make_identity(nc, identB)

        xt = big.tile([128, NE, S], BF16)  # x^T; cols [0, HALF) = own q-rows
        wt = big.tile([128, NE, 384], BF16)  # [WkT|WqT | WkT|WvT | WvT | WqT]
        qk = big.tile([128, HALF], BF16)  # rows 0-63 K^T own, 64-127 Q^T own
        kv = big.tile([128, HALF], BF16)  # rows 0-63 K^T oth, 64-127 V^T oth
        qts = big.tile([64, HALF], BF16)  # Q^T shifted to base partition 0
        vto = big.tile([64, HALF], BF16)  # V^T other shifted to base part. 0
        vt = big.tile([65, HALF], BF16)  # V^T own; row 64 = ones
        vn = big.tile([128, NKT, D + 1], BF16)  # V natural + ones column
        att_sb = big.tile([65, HALF], BF16)  # attn^T + denominator row
        # odd K^T tiles shifted to base partition 64 for pass-1 row
        # tiling (rows 64-127: own tiles 1,3..15 / oth tiles 17,19..31)
        ko = big.tile([128, 1024], BF16)
        kq = big.tile([128, 1024], BF16)

        # --- PE warm-up: the HAM duty controller halves the PE clock after
        # ~2.5us of idleness and takes 5-13us at half clock to restore full
        # duty, so keep the PE streaming junk matmuls from the instant its
        # preamble ends until the first input DMAs land.
        warm = psA.tile([128, 512], FP32, tag="ps")
        for _ in range(N_WARM):
            nc.tensor.matmul(
                out=warm[0:128, 0:512],
                lhsT=junk[:, 0:128],
                rhs=junk[:, :],
                start=True,
                stop=True,
            )

        # --- input DMAs: ALL on the sync queue, issued in consumption
        # order. A single queue drains FIFO across all 16 DMA engines, so
        # arrival order == issue order and the PE never waits on a piece
        # that lost a queue-arbitration race (the old 3-queue split left
        # 12.6us of mid-ramp PE idle + HAM re-throttles). Measured stream
        # rate ~0.32 MB/us: wt ~9.5us, own 0:512 ~12.8, 512:1024 ~16.1,
        # own 1024:2048 ~22.7, oth chunks ~26/29/33/36us.
        xq_r = xt_q_d.rearrange("(t p) s -> p t s", p=128)
        xo_r = xt_o_d.rearrange("(t p) s -> p t s", p=128)
        wt_r = wt_d.rearrange("(t p) d -> p t d", p=128)
        # wt and own cols 0:512 interleaved per e-tile PAIR so the fused
        # A/C/Q prologue's et-pair k needs only the first 2k+2 pieces —
        # first useful matmul ~3us earlier than with a monolithic wt load.
        # Pieces stay >=192KB: below ~256KB the ~625ns-per-start HWDGE
        # config rate on sync throttles the stream under the ~0.36MB/us
        # the HBM path sustains.
        for et in range(0, NE, 2):
            nc.sync.dma_start(out=wt[:, et : et + 2, :], in_=wt_r[:, et : et + 2, :])
            nc.sync.dma_start(
                out=xt[:, et : et + 2, 0:512], in_=xq_r[:, et : et + 2, 0:512]
            )
        for et in range(0, NE, 2):
            nc.sync.dma_start(
                out=xt[:, et : et + 2, 512:1024], in_=xq_r[:, et : et + 2, 512:1024]
            )
        nc.sync.dma_start(out=xt[:, :, 1024:2048], in_=xq_r[:, :, 1024:2048])
        for c in range(4):
            nc.sync.dma_start(
                out=xt[:, :, HALF + c * 512 : HALF + (c + 1) * 512],
                in_=xo_r[:, :, c * 512 : (c + 1) * 512],
            )

        nc.vector.memset(vt[64:65, :], 1.0)

        # one packed projection half-chunk of 512 cols
        def proj_half(w0, wm, dst, src_x0, d0):
            acc = psA.tile([128, 512], FP32, tag="ps")
            for et in range(NE):
                nc.tensor.matmul(
                    out=acc[0:wm, 0:512],
                    lhsT=wt[:, et, w0 : w0 + wm],
                    rhs=xt[:, et, src_x0 : src_x0 + 512],
                    start=(et == 0),
                    stop=(et == NE - 1),
                )
            nc.vector.tensor_copy(out=dst[:, d0 : d0 + 512], in_=acc[0:wm, 0:512])

        def shift(dst, src, d0):
            # gpsimd's SWDGE queue is otherwise idle (inputs all ride the
            # sync queue now), so shifts never wait behind bulk input
            # pieces and don't steal ScalarE time from the exps.
            nc.gpsimd.dma_start(
                out=dst[:, d0 : d0 + 512], in_=src[64:128, d0 : d0 + 512]
            )

        def shift_odd(dst, src):
            # pack the odd 128-col K^T tiles of `src` rows 0:64 into
            # `dst` rows 64:128 (base partition 64) for pass-1 row tiling
            nc.gpsimd.dma_start(
                out=dst[64:128, :].rearrange("p (t c) -> p t c", c=128),
                in_=src[0:64, :].rearrange("p (t c) -> p t c", c=128)[:, 1::2, :],
            )

        def v_transpose(k):
            tp = psA.tile([128, 512], BF16, tag="ps")
            if k < 16:  # own half: vt carries the ones row
                nc.tensor.transpose(
                    out=tp[0:128, 0:65],
                    in_=vt[:, k * 128 : (k + 1) * 128],
                    identity=identB[0:65, 0:65],
                )
                nc.vector.tensor_copy(out=vn[:, k, :], in_=tp[0:128, 0:65])
            else:  # other half: V^T shifted into vto (base partition 0)
                j = k - 16
                nc.tensor.transpose(
                    out=tp[0:128, 0:64],
                    in_=vto[:, j * 128 : (j + 1) * 128],
                    identity=identB[0:64, 0:64],
                )
                nc.vector.memset(vn[:, k, D : D + 1], 1.0)
                nc.vector.tensor_copy(out=vn[:, k, 0:D], in_=tp[0:128, 0:64])

        # fused A/C(/Q) projection lump for one 512-col chunk of own
        # q-rows. A gives [K^T;Q^T] packed; Q (prologue chunks only)
        # produces the base-0 Q^T copy directly, replacing a shift-DMA
        # whose ~5.5us latency (contending with the input stream for DMA
        # engines) sat on the scores-iter-0 critical path. (A col-tiled
        # K64 projection for free K^T@64 was tried: the framework pairs
        # every matmul with its own LDWEIGHTS, which cannot pull ahead
        # past an in-flight matmul on overlapping PE rows, so the
        # "concurrent" col-tiles serialize and cost full price.)
        def ack_lump(hh, with_q):
            x0 = hh * 512
            accA = psA.tile([128, 512], FP32, tag="ps")
            accC = psA.tile([128, 512], FP32, tag="ps")
            if with_q:
                accQ = psA.tile([128, 512], FP32, tag="ps")
            for et in range(NE):
                fl = dict(start=(et == 0), stop=(et == NE - 1))
                rhs = xt[:, et, x0 : x0 + 512]
                nc.tensor.matmul(
                    out=accA[0:128, 0:512], lhsT=wt[:, et, 0:128], rhs=rhs, **fl
                )
                nc.tensor.matmul(
                    out=accC[0:64, 0:512], lhsT=wt[:, et, 256:320], rhs=rhs, **fl
                )
                if with_q:
                    nc.tensor.matmul(
                        out=accQ[0:64, 0:512], lhsT=wt[:, et, 320:384], rhs=rhs, **fl
                    )
            if with_q:
                # qts gates scores iter 0: copy it first
                nc.vector.tensor_copy(
                    out=qts[:, x0 : x0 + 512], in_=accQ[0:64, 0:512]
                )
            nc.vector.tensor_copy(out=qk[:, x0 : x0 + 512], in_=accA[0:128, 0:512])
            nc.vector.tensor_copy(out=vt[0:64, x0 : x0 + 512], in_=accC[0:64, 0:512])

        # --- prologue: chunks 0-1 (own q-cols 0:1024), consumed et-by-et
        # as the pieces land ---
        for hh in range(2):
            ack_lump(hh, with_q=True)

        # pass-0 side-slot schedule: iter k -> own-chunk ACK lump or oth
        # chunk B projection. Slots track the ordered-queue arrivals (own
        # 1024:2048 by ~24us, oth chunk c by ~27+3c); deadlines: ACK hh
        # feeds scores k=4hh, B-lump hh feeds scores k=16+4hh. The ko/kq
        # odd-tile shifts and qts shifts only feed pass 1 (loose), and
        # the attn lag-5 backlog bridges the arrival waits.
        SIDE = {4: ("A", 2), 5: ("C", 2), 6: ("A", 3), 7: ("C", 3),
                9: ("B", 0), 11: ("B", 1), 13: ("B", 2), 15: ("B", 3)}

        def side_work(k):
            s = SIDE.get(k)
            if s is not None:
                kind, hh = s
                if kind == "A":
                    proj_half(0, 128, qk, hh * 512, hh * 512)
                    shift(qts, qk, hh * 512)
                    if hh == 3:
                        shift_odd(ko, qk)
                elif kind == "C":
                    proj_half(256, 64, vt[0:64, :], hh * 512, hh * 512)
                else:
                    proj_half(128, 128, kv, HALF + hh * 512, hh * 512)
                    shift(vto, kv, hh * 512)
                    if hh == 3:
                        shift_odd(kq, kv)
            if k == 2:
                v_transpose(0)
                v_transpose(1)
            elif k >= 3:
                v_transpose(k - 1)
                if k == NKT - 1:
                    v_transpose(NKT - 1)

        out_engs = [nc.sync, nc.scalar]

        def exp_half(sc, p, c, eng):
            # eng 'S': exact ACT exp on ScalarE. 'V': one-instruction
            # Schraudolph bit-trick on VectorE (see module docstring).
            dst = p[:, c * 512 : (c + 1) * 512]
            if eng == "S":
                nc.scalar.activation(out=dst, in_=sc[:, :], func=Exp, scale=SCALE)
            else:
                nc.vector.tensor_scalar(
                    dst.bitcast(I16), sc[:, :], A16, B16, Mult, Add
                )

        def ship(ps, att_ps):
            # ship this pass's attn^T + denominators in bf16 (host
            # normalizes in fp32); the c0/c1 copies ride different
            # engines so the tail's two copy+DMA chains overlap.
            for c in range(2):
                cols = slice(ps * 1024 + c * 512, ps * 1024 + (c + 1) * 512)
                pcols = slice(c * 512, (c + 1) * 512)
                if c == 0:
                    nc.vector.tensor_copy(
                        out=att_sb[:, cols], in_=att_ps[0:65, pcols]
                    )
                else:
                    nc.scalar.activation(
                        out=att_sb[:, cols], in_=att_ps[0:65, pcols], func=Cpy
                    )
                out_engs[c].dma_start(out=out_d[:, cols], in_=att_sb[:, cols])

        # row-tiled score pair for k-tiles (2j, 2j+1): the even tile
        # (K^T at base partition 0, Q^T copy at base 0) and the odd tile
        # (K^T at base 64, Q^T at base 64 straight out of the packed
        # A-projection) occupy disjoint PE row-groups, so their matmuls
        # run concurrently (measured dstart ~4ns): scores cost halves.
        def score_pair(j, q_base, klhs_e, klhs_o):
            p_e = pp.tile([128, 1024], BF16)
            p_o = pp.tile([128, 1024], BF16)
            for c in range(2):
                q0 = q_base + c * 512
                sc_e = psA.tile([128, 512], FP32, tag="ps")
                sc_o = psA.tile([128, 512], FP32, tag="ps")
                nc.tensor.matmul(
                    out=sc_e[:, :], lhsT=klhs_e, rhs=qts[:, q0 : q0 + 512],
                    start=True, stop=True,
                )
                nc.tensor.matmul(
                    out=sc_o[:, :], lhsT=klhs_o, rhs=qk[64:128, q0 : q0 + 512],
                    start=True, stop=True,
                )
                exp_half(sc_e, p_e, c, "SV"[c])
                exp_half(sc_o, p_o, c, "VS"[c])
            p_tiles[2 * j] = p_e
            p_tiles[2 * j + 1] = p_o

        # --- pass 0: q-cols 0:1024, plain scores + side work. The attn
        # lag-5 backlog doubles as a work reservoir bridging input-piece
        # arrival waits; the balanced per-half S/V exp split keeps either
        # engine under the iteration wall.
        att_ps = psB.tile([128, 1024], FP32)
        p_tiles = {}
        nxt = [0]

        def drain(upto):
            while nxt[0] <= upto:
                _attn(nc, att_ps, vn, p_tiles, nxt[0])
                nxt[0] += 1

        for k in range(NKT):
            if k < 16:
                klhs = qk[0:64, k * 128 : (k + 1) * 128]
            else:
                klhs = kv[0:64, (k - 16) * 128 : (k - 15) * 128]
            p = pp.tile([128, 1024], BF16)
            for c in range(2):
                sc = psA.tile([128, 512], FP32, tag="ps")
                nc.tensor.matmul(
                    out=sc[:, :], lhsT=klhs, rhs=qts[:, c * 512 : (c + 1) * 512],
                    start=True, stop=True,
                )
                exp_half(sc, p, c, "SV"[(k + c) % 2])
            p_tiles[k] = p
            side_work(k)
            drain(k - 5)
        drain(NKT - 1)
        ship(0, att_ps)

        # --- pass 1: q-cols 1024:2048, all 16 pairs row-tiled (odd
        # tiles' K^T from the ko/kq shifts done during pass 0)
        att_ps = psB.tile([128, 1024], FP32)
        nxt = [0]
        for j in range(NKT // 2):
            ke = 2 * j
            if j < 8:
                klhs_e = qk[0:64, ke * 128 : (ke + 1) * 128]
                klhs_o = ko[64:128, j * 128 : (j + 1) * 128]
            else:
                klhs_e = kv[0:64, (ke - 16) * 128 : (ke - 15) * 128]
                klhs_o = kq[64:128, (j - 8) * 128 : (j - 7) * 128]
            score_pair(j, 1024, klhs_e, klhs_o)
            drain(ke + 1 - 5)
        drain(NKT - 1)
        ship(1, att_ps)

    nc.compile()
    _CACHE["nc"] = nc
    return nc


def _attn(nc, att_ps, vn, p_tiles, k):
    p = p_tiles.pop(k)
    for c in range(2):
        nc.tensor.matmul(
            out=att_ps[0:65, c * 512 : (c + 1) * 512],
            lhsT=vn[:, k, :],
            rhs=p[:, c * 512 : (c + 1) * 512],
            start=(k == 0),
            stop=(k == NKT - 1),
            skip_group_check=True,
        )


def _make_in_maps(x, Wq, Wk, Wv):
    import ml_dtypes

    bf16 = ml_dtypes.bfloat16
    xT = np.ascontiguousarray(x.transpose(0, 2, 1)).astype(bf16)  # [B, E, S]
    wt = np.concatenate(
        [Wk.T, Wq.T, Wk.T, Wv.T, Wv.T, Wq.T], axis=1
    ).astype(bf16)  # [E, 384]
    in_maps = []
    for c in range(N_CORES):
        b, h = divmod(c, 2)
        in_maps.append(
            {
                "xt_q": np.ascontiguousarray(xT[b, :, h * HALF : (h + 1) * HALF]),
                "xt_o": np.ascontiguousarray(
                    xT[b, :, (1 - h) * HALF : (2 - h) * HALF]
                ),
                "wt": wt,
            }
        )
    return in_maps


def _run(x, Wq, Wk, Wv, trace=False):
    from concourse.bass_utils import run_bass_kernel_spmd

    nc = _build()
    in_maps = _make_in_maps(x, Wq, Wk, Wv)
    res = run_bass_kernel_spmd(
        nc, in_maps, core_ids=list(range(N_CORES)), trace=trace
    )
    out = np.empty((B, S, D), dtype=np.float32)
    for c in range(N_CORES):
        b, h = divmod(c, 2)
        att = np.asarray(
            res.results[c]["out"], dtype=np.float32
        )  # [65, HALF]: attn^T rows + denom row (bf16 on the wire)
        out[b, h * HALF : (h + 1) * HALF, :] = (att[0:D] / att[D : D + 1]).T
    return out, res


def kernel(x, Wq, Wk, Wv):
    out, _ = _run(
        np.asarray(x, dtype=np.float32),
        np.asarray(Wq, dtype=np.float32),
        np.asarray(Wk, dtype=np.float32),
        np.asarray(Wv, dtype=np.float32),
    )
    return out



# revision 48
# speedup vs baseline: 1.2165x; 1.0021x over previous
"""Single-head attention (B=4, S=4096, E=1024, D=64) on 8 TRN2 NeuronCores.

Sharding: data-parallel over (batch, query-half): core c handles batch
b = c // 2 and query rows [h*2048, (h+1)*2048) with h = c % 2. Each core
computes Q for its own 2048 rows and K/V for the full 4096 rows of its
batch (inputs are shipped host-pretransposed per half).

Per-core dataflow (TensorE matmuls in bf16; fp32 accumulation in PSUM).
Projections pack weight matrices into wide stationary operands:
  qk [128, 2048] = [K^T_own; Q^T_own]   (lump A, lhsT = [WkT | WqT])
  kv [128, 2048] = [K^T_oth; V^T_oth]   (lump B, lhsT = [WkT | WvT])
  vt [65, 2048]  = V^T_own + ones row   (lump C, lhsT = WvT)
  qts [64, 2048] = Q^T at base partition 0 (prologue lump Q, lhsT = WqT;
      a direct projection because the SBUF-SBUF shift-DMA alternative has
      ~5.5us latency under input-stream contention and sat on the
      scores-iter-0 critical path; pass-1 columns DO use shift-DMAs since
      their deadline is a whole pass away)
  scores^T[k, q] = K^T.T @ Q^T -> exp -> P bf16
  attn^T[65, q] += V_aug.T @ P   (row 64 accumulates softmax denominators)
  output = attn^T + denominators in bf16; host transposes + normalizes.

Input DMA rides ONE queue (sync HWDGE) in strict consumption order --
wt/x interleaved per e-tile pair, then own cols 512:1024, own 1024:2048,
then the other half in 512-col chunks. A single queue drains FIFO across
all 16 DMA engines at ~0.36MB/us, so arrival order == issue order and the
PE never waits on a piece that lost a queue-arbitration race. Pieces stay
>=192KB so the ~625ns-per-start config rate keeps ahead of the stream.
The fused A/C/Q prologue consumes pieces et-by-et as they land; the
pass-0 side-slot schedule (A2/C2/A3/C3 then B0-B3) tracks arrivals, and
the attn lag-5 backlog bridges residual arrival jitter.

The scores matmul contracts over only D=64 of the PE's 128 rows, so
pass 1 runs k-tiles as ROW-TILED PAIRS: the even tile (K^T at base
partition 0, streaming the qts copy) and the odd tile (K^T shifted to
base 64 during pass 0, streaming Q^T straight from qk rows 64:127)
occupy disjoint PE row-groups and their matmuls run concurrently
(measured dstart ~4ns) -- scores throughput doubles. Pass 0 stays plain:
its pace is set by side-lump projections and input arrival anyway, and
the base-64 K^T copies do not exist yet.

The exp is split per 512-col half across both engines (ScalarE exact ACT
exp ~0.6us/half; VectorE one-instruction Schraudolph bit-trick:
i16 = rne(x*A + B) bitcast to bf16 ~= exp(SCALE*x), ~1.8% sawtooth whose
mean bias cancels in the softmax ratio; on half the weights it adds
~0.7% output rel err). PSUM: six single-bank [128,512] score/side slots
+ the 2-bank attn accumulator fill all 8 banks.

The HAM duty controller halves the PE clock after ~3.4us of idleness and
takes ~3.4us of sustained work to re-grant full duty, so junk warm-up
matmuls (gated on a gpsimd memset) keep the PE streaming from preamble
end until the first input pieces land; with the ordered DMA there are no
mid-kernel idle windows and the PE stays at full clock throughout.
"""

import numpy as np

B, S, E, D = 4, 4096, 1024, 64
HALF = S // 2
N_CORES = 8
SCALE = 1.0 / np.sqrt(D)

NE = E // 128  # 8 e-tiles
NKT = S // 128  # 32 k-tiles
N_WARM = 11  # 512-col PE warm-up matmuls covering the preamble + DMA wait

# Schraudolph exp-approx constants (bf16 bit pattern via int16):
#   i16 = round(x * A16 + B16); bitcast bf16 ~= exp(SCALE * x)
LOG2E = 1.4426950408889634
A16 = SCALE * 128.0 * LOG2E
B16 = 127.0 * 128.0 - 7.3


_CACHE = {}


def _build():
    if "nc" in _CACHE:
        return _CACHE["nc"]

    from contextlib import ExitStack

    import concourse.bacc as bacc
    import concourse.tile as tile
    from concourse import mybir
    from concourse.masks import make_identity

    FP32 = mybir.dt.float32
    BF16 = mybir.dt.bfloat16
    I16 = mybir.dt.int16
    Exp = mybir.ActivationFunctionType.Exp
    Cpy = mybir.ActivationFunctionType.Copy
    Mult = mybir.AluOpType.mult
    Add = mybir.AluOpType.add

    nc = bacc.Bacc(
        "TRN2", target_bir_lowering=False, debug=False, num_devices=N_CORES
    )

    xt_q_d = nc.dram_tensor("xt_q", [E, HALF], BF16, kind="ExternalInput").ap()
    xt_o_d = nc.dram_tensor("xt_o", [E, HALF], BF16, kind="ExternalInput").ap()
    wt_d = nc.dram_tensor("wt", [E, 384], BF16, kind="ExternalInput").ap()
    out_d = nc.dram_tensor("out", [D + 1, HALF], BF16, kind="ExternalOutput").ap()

    with tile.TileContext(nc) as tc, ExitStack() as ctx:
        const = ctx.enter_context(tc.tile_pool(name="const", bufs=1))
        big = ctx.enter_context(tc.tile_pool(name="big", bufs=1))
        # P tiles: each [128,1024] tile gets its c0/c1 halves written by
        # DIFFERENT exp engines (balanced S/V split), so one pool is fine —
        # slot reuse gates both engines on the same attn read.
        pp = ctx.enter_context(tc.tile_pool(name="pp", bufs=7))
        # psA: six single-bank [128,512] score/side slots; psB: the 2-bank
        # attn accumulator. 6*2KB + 4KB = 16KB = all 8 PSUM banks.
        psA = ctx.enter_context(tc.tile_pool(name="psA", bufs=6, space="PSUM"))
        psB = ctx.enter_context(tc.tile_pool(name="psB", bufs=1, space="PSUM"))

        identB = const.tile([128, 128], BF16)
        junk = const.tile([128, 512], BF16)
        # memset on gpsimd: its preamble ends ~1us before vector's, so
        # the PE warm-up starts that much earlier.
        nc.gpsimd.memset(junk[:, :], 0.0)
        make_identity(nc, identB)

        xt = big.tile([128, NE, S], BF16)  # x^T; cols [0, HALF) = own q-rows
        wt = big.tile([128, NE, 384], BF16)  # [WkT|WqT | WkT|WvT | WvT | WqT]
        qk = big.tile([128, HALF], BF16)  # rows 0-63 K^T own, 64-127 Q^T own
        kv = big.tile([128, HALF], BF16)  # rows 0-63 K^T oth, 64-127 V^T oth
        qts = big.tile([64, HALF], BF16)  # Q^T shifted to base partition 0
        vto = big.tile([64, HALF], BF16)  # V^T other shifted to base part. 0
        vt = big.tile([65, HALF], BF16)  # V^T own; row 64 = ones
        vn = big.tile([128, NKT, D + 1], BF16)  # V natural + ones column
        att_sb = big.tile([65, HALF], BF16)  # attn^T + denominator row
        # odd K^T tiles shifted to base partition 64 for pass-1 row
        # tiling (rows 64-127: own tiles 1,3..15 / oth tiles 17,19..31)
        ko = big.tile([128, 1024], BF16)
        kq = big.tile([128, 1024], BF16)

        # --- PE warm-up: the HAM duty controller halves the PE clock after
        # ~2.5us of idleness and takes 5-13us at half clock to restore full
        # duty, so keep the PE streaming junk matmuls from the instant its
        # preamble ends until the first input DMAs land.
        warm = psA.tile([128, 512], FP32, tag="ps")
        for _ in range(N_WARM):
            nc.tensor.matmul(
                out=warm[0:128, 0:512],
                lhsT=junk[:, 0:128],
                rhs=junk[:, :],
                start=True,
                stop=True,
            )

        # --- input DMAs: ALL on the sync queue, issued in consumption
        # order. A single queue drains FIFO across all 16 DMA engines, so
        # arrival order == issue order and the PE never waits on a piece
        # that lost a queue-arbitration race (the old 3-queue split left
        # 12.6us of mid-ramp PE idle + HAM re-throttles). Measured stream
        # rate ~0.32 MB/us: wt ~9.5us, own 0:512 ~12.8, 512:1024 ~16.1,
        # own 1024:2048 ~22.7, oth chunks ~26/29/33/36us.
        xq_r = xt_q_d.rearrange("(t p) s -> p t s", p=128)
        xo_r = xt_o_d.rearrange("(t p) s -> p t s", p=128)
        wt_r = wt_d.rearrange("(t p) d -> p t d", p=128)
        # wt and own cols 0:512 interleaved per e-tile PAIR so the fused
        # A/C/Q prologue's et-pair k needs only the first 2k+2 pieces —
        # first useful matmul ~3us earlier than with a monolithic wt load.
        # Pieces stay >=192KB: below ~256KB the ~625ns-per-start HWDGE
        # config rate on sync throttles the stream under the ~0.36MB/us
        # the HBM path sustains.
        for et in range(0, NE, 2):
            nc.sync.dma_start(out=wt[:, et : et + 2, :], in_=wt_r[:, et : et + 2, :])
            nc.sync.dma_start(
                out=xt[:, et : et + 2, 0:512], in_=xq_r[:, et : et + 2, 0:512]
            )
        for et in range(0, NE, 2):
            nc.sync.dma_start(
                out=xt[:, et : et + 2, 512:1024], in_=xq_r[:, et : et + 2, 512:1024]
            )
        nc.sync.dma_start(out=xt[:, :, 1024:2048], in_=xq_r[:, :, 1024:2048])
        for c in range(4):
            nc.sync.dma_start(
                out=xt[:, :, HALF + c * 512 : HALF + (c + 1) * 512],
                in_=xo_r[:, :, c * 512 : (c + 1) * 512],
            )

        nc.vector.memset(vt[64:65, :], 1.0)

        # one packed projection half-chunk of 512 cols
        def proj_half(w0, wm, dst, src_x0, d0):
            acc = psA.tile([128, 512], FP32, tag="ps")
            for et in range(NE):
                nc.tensor.matmul(
                    out=acc[0:wm, 0:512],
                    lhsT=wt[:, et, w0 : w0 + wm],
                    rhs=xt[:, et, src_x0 : src_x0 + 512],
                    start=(et == 0),
                    stop=(et == NE - 1),
                )
            nc.vector.tensor_copy(out=dst[:, d0 : d0 + 512], in_=acc[0:wm, 0:512])

        def shift(dst, src, d0):
            # gpsimd's SWDGE queue is otherwise idle (inputs all ride the
            # sync queue now), so shifts never wait behind bulk input
            # pieces and don't steal ScalarE time from the exps.
            nc.gpsimd.dma_start(
                out=dst[:, d0 : d0 + 512], in_=src[64:128, d0 : d0 + 512]
            )

        def shift_odd(dst, src):
            # pack the odd 128-col K^T tiles of `src` rows 0:64 into
            # `dst` rows 64:128 (base partition 64) for pass-1 row tiling
            nc.gpsimd.dma_start(
                out=dst[64:128, :].rearrange("p (t c) -> p t c", c=128),
                in_=src[0:64, :].rearrange("p (t c) -> p t c", c=128)[:, 1::2, :],
            )

        def v_transpose(k):
            tp = psA.tile([128, 512], BF16, tag="ps")
            if k < 16:  # own half: vt carries the ones row
                nc.tensor.transpose(
                    out=tp[0:128, 0:65],
                    in_=vt[:, k * 128 : (k + 1) * 128],
                    identity=identB[0:65, 0:65],
                )
                nc.vector.tensor_copy(out=vn[:, k, :], in_=tp[0:128, 0:65])
            else:  # other half: V^T shifted into vto (base partition 0)
                j = k - 16
                nc.tensor.transpose(
                    out=tp[0:128, 0:64],
                    in_=vto[:, j * 128 : (j + 1) * 128],
                    identity=identB[0:64, 0:64],
                )
                nc.vector.memset(vn[:, k, D : D + 1], 1.0)
                nc.vector.tensor_copy(out=vn[:, k, 0:D], in_=tp[0:128, 0:64])

        # fused A/C(/Q) projection lump for one 512-col chunk of own
        # q-rows. A gives [K^T;Q^T] packed; Q (prologue chunks only)
        # produces the base-0 Q^T copy directly, replacing a shift-DMA
        # whose ~5.5us latency (contending with the input stream for DMA
        # engines) sat on the scores-iter-0 critical path. (A col-tiled
        # K64 projection for free K^T@64 was tried: the framework pairs
        # every matmul with its own LDWEIGHTS, which cannot pull ahead
        # past an in-flight matmul on overlapping PE rows, so the
        # "concurrent" col-tiles serialize and cost full price.)
        def ack_lump(hh, with_q):
            x0 = hh * 512
            accA = psA.tile([128, 512], FP32, tag="ps")
            accC = psA.tile([128, 512], FP32, tag="ps")
            if with_q:
                accQ = psA.tile([128, 512], FP32, tag="ps")
            for et in range(NE):
                fl = dict(start=(et == 0), stop=(et == NE - 1))
                rhs = xt[:, et, x0 : x0 + 512]
                nc.tensor.matmul(
                    out=accA[0:128, 0:512], lhsT=wt[:, et, 0:128], rhs=rhs, **fl
                )
                nc.tensor.matmul(
                    out=accC[0:64, 0:512], lhsT=wt[:, et, 256:320], rhs=rhs, **fl
                )
                if with_q:
                    nc.tensor.matmul(
                        out=accQ[0:64, 0:512], lhsT=wt[:, et, 320:384], rhs=rhs, **fl
                    )
            if with_q:
                # qts gates scores iter 0: copy it first
                nc.vector.tensor_copy(
                    out=qts[:, x0 : x0 + 512], in_=accQ[0:64, 0:512]
                )
            nc.vector.tensor_copy(out=qk[:, x0 : x0 + 512], in_=accA[0:128, 0:512])
            nc.vector.tensor_copy(out=vt[0:64, x0 : x0 + 512], in_=accC[0:64, 0:512])

        # --- prologue: chunks 0-1 (own q-cols 0:1024), consumed et-by-et
        # as the pieces land ---
        for hh in range(2):
            ack_lump(hh, with_q=True)

        # pass-0 side-slot schedule: iter k -> own-chunk ACK lump or oth
        # chunk B projection. Slots track the ordered-queue arrivals (own
        # 1024:2048 by ~24us, oth chunk c by ~27+3c); deadlines: ACK hh
        # feeds scores k=4hh, B-lump hh feeds scores k=16+4hh. The ko/kq
        # odd-tile shifts and qts shifts only feed pass 1 (loose), and
        # the attn lag-5 backlog bridges the arrival waits.
        SIDE = {5: ("A", 2), 6: ("C", 2), 7: ("A", 3), 8: ("C", 3),
                10: ("B", 0), 12: ("B", 1), 14: ("B", 2), 16: ("B", 3)}

        def side_work(k):
            s = SIDE.get(k)
            if s is not None:
                kind, hh = s
                if kind == "A":
                    proj_half(0, 128, qk, hh * 512, hh * 512)
                    shift(qts, qk, hh * 512)
                    if hh == 3:
                        shift_odd(ko, qk)
                elif kind == "C":
                    proj_half(256, 64, vt[0:64, :], hh * 512, hh * 512)
                else:
                    proj_half(128, 128, kv, HALF + hh * 512, hh * 512)
                    shift(vto, kv, hh * 512)
                    if hh == 3:
                        shift_odd(kq, kv)
            if k == 2:
                v_transpose(0)
                v_transpose(1)
            elif k >= 3:
                v_transpose(k - 1)
                if k == NKT - 1:
                    v_transpose(NKT - 1)

        out_engs = [nc.sync, nc.scalar]

        def exp_half(sc, p, c, eng):
            # eng 'S': exact ACT exp on ScalarE. 'V': one-instruction
            # Schraudolph bit-trick on VectorE (see module docstring).
            dst = p[:, c * 512 : (c + 1) * 512]
            if eng == "S":
                nc.scalar.activation(out=dst, in_=sc[:, :], func=Exp, scale=SCALE)
            else:
                nc.vector.tensor_scalar(
                    dst.bitcast(I16), sc[:, :], A16, B16, Mult, Add
                )

        def ship_half(ps, att_ps, c):
            # ship one 512-col half of this pass's attn^T + denominators
            # in bf16 (host normalizes in fp32); the c0/c1 copies ride
            # different engines so the two copy+DMA chains overlap.
            cols = slice(ps * 1024 + c * 512, ps * 1024 + (c + 1) * 512)
            pcols = slice(c * 512, (c + 1) * 512)
            if c == 0:
                nc.vector.tensor_copy(out=att_sb[:, cols], in_=att_ps[0:65, pcols])
            else:
                nc.scalar.activation(
                    out=att_sb[:, cols], in_=att_ps[0:65, pcols], func=Cpy
                )
            out_engs[c].dma_start(out=out_d[:, cols], in_=att_sb[:, cols])

        def ship(ps, att_ps):
            ship_half(ps, att_ps, 0)
            ship_half(ps, att_ps, 1)

        # row-tiled score pair for k-tiles (2j, 2j+1): the even tile
        # (K^T at base partition 0, Q^T copy at base 0) and the odd tile
        # (K^T at base 64, Q^T at base 64 straight out of the packed
        # A-projection) occupy disjoint PE row-groups, so their matmuls
        # run concurrently (measured dstart ~4ns): scores cost halves.
        def score_pair(j, q_base, klhs_e, klhs_o):
            p_e = pp.tile([128, 1024], BF16)
            p_o = pp.tile([128, 1024], BF16)
            for c in range(2):
                q0 = q_base + c * 512
                sc_e = psA.tile([128, 512], FP32, tag="ps")
                sc_o = psA.tile([128, 512], FP32, tag="ps")
                nc.tensor.matmul(
                    out=sc_e[:, :], lhsT=klhs_e, rhs=qts[:, q0 : q0 + 512],
                    start=True, stop=True,
                )
                nc.tensor.matmul(
                    out=sc_o[:, :], lhsT=klhs_o, rhs=qk[64:128, q0 : q0 + 512],
                    start=True, stop=True,
                )
                exp_half(sc_e, p_e, c, "SV"[c])
                exp_half(sc_o, p_o, c, "VS"[c])
            p_tiles[2 * j] = p_e
            p_tiles[2 * j + 1] = p_o

        # --- pass 0: q-cols 0:1024, plain scores + side work. The attn
        # lag-5 backlog doubles as a work reservoir bridging input-piece
        # arrival waits; the balanced per-half S/V exp split keeps either
        # engine under the iteration wall.
        att_ps = psB.tile([128, 1024], FP32)
        p_tiles = {}
        nxt = [0]

        def drain(upto):
            while nxt[0] <= upto:
                _attn(nc, att_ps, vn, p_tiles, nxt[0])
                nxt[0] += 1

        for k in range(NKT):
            if k < 16:
                klhs = qk[0:64, k * 128 : (k + 1) * 128]
            else:
                klhs = kv[0:64, (k - 16) * 128 : (k - 15) * 128]
            p = pp.tile([128, 1024], BF16)
            for c in range(2):
                sc = psA.tile([128, 512], FP32, tag="ps")
                nc.tensor.matmul(
                    out=sc[:, :], lhsT=klhs, rhs=qts[:, c * 512 : (c + 1) * 512],
                    start=True, stop=True,
                )
                exp_half(sc, p, c, "SV"[(k + c) % 2])
            p_tiles[k] = p
            side_work(k)
            drain(k - 5)
        drain(NKT - 1)
        ship(0, att_ps)

        # --- pass 1: q-cols 1024:2048, all 16 pairs row-tiled (odd
        # tiles' K^T from the ko/kq shifts done during pass 0)
        att_ps = psB.tile([128, 1024], FP32)
        nxt = [0]
        for j in range(NKT // 2):
            ke = 2 * j
            if j < 8:
                klhs_e = qk[0:64, ke * 128 : (ke + 1) * 128]
                klhs_o = ko[64:128, j * 128 : (j + 1) * 128]
            else:
                klhs_e = kv[0:64, (ke - 16) * 128 : (ke - 15) * 128]
                klhs_o = kq[64:128, (j - 8) * 128 : (j - 7) * 128]
            score_pair(j, 1024, klhs_e, klhs_o)
            drain(ke + 1 - 5)
        # drain the attn backlog column-major and ship each finished
        # column immediately: the c0 copy+DMA chain starts ~1us earlier
        # and fully overlaps the c1 matmuls + chain in the tail.
        rem = list(range(nxt[0], NKT))
        for c in range(2):
            for kt in rem:
                nc.tensor.matmul(
                    out=att_ps[0:65, c * 512 : (c + 1) * 512],
                    lhsT=vn[:, kt, :],
                    rhs=p_tiles[kt][:, c * 512 : (c + 1) * 512],
                    start=False,
                    stop=(kt == NKT - 1),
                    skip_group_check=True,
                )
            ship_half(1, att_ps, c)
        for kt in rem:
            p_tiles.pop(kt)

    nc.compile()
    _CACHE["nc"] = nc
    return nc


def _attn(nc, att_ps, vn, p_tiles, k):
    p = p_tiles.pop(k)
    for c in range(2):
        nc.tensor.matmul(
            out=att_ps[0:65, c * 512 : (c + 1) * 512],
            lhsT=vn[:, k, :],
            rhs=p[:, c * 512 : (c + 1) * 512],
            start=(k == 0),
            stop=(k == NKT - 1),
            skip_group_check=True,
        )


def _make_in_maps(x, Wq, Wk, Wv):
    import ml_dtypes

    bf16 = ml_dtypes.bfloat16
    xT = np.ascontiguousarray(x.transpose(0, 2, 1)).astype(bf16)  # [B, E, S]
    wt = np.concatenate(
        [Wk.T, Wq.T, Wk.T, Wv.T, Wv.T, Wq.T], axis=1
    ).astype(bf16)  # [E, 384]
    in_maps = []
    for c in range(N_CORES):
        b, h = divmod(c, 2)
        in_maps.append(
            {
                "xt_q": np.ascontiguousarray(xT[b, :, h * HALF : (h + 1) * HALF]),
                "xt_o": np.ascontiguousarray(
                    xT[b, :, (1 - h) * HALF : (2 - h) * HALF]
                ),
                "wt": wt,
            }
        )
    return in_maps


def _run(x, Wq, Wk, Wv, trace=False):
    from concourse.bass_utils import run_bass_kernel_spmd

    nc = _build()
    in_maps = _make_in_maps(x, Wq, Wk, Wv)
    res = run_bass_kernel_spmd(
        nc, in_maps, core_ids=list(range(N_CORES)), trace=trace
    )
    out = np.empty((B, S, D), dtype=np.float32)
    for c in range(N_CORES):
        b, h = divmod(c, 2)
        att = np.asarray(
            res.results[c]["out"], dtype=np.float32
        )  # [65, HALF]: attn^T rows + denom row (bf16 on the wire)
        out[b, h * HALF : (h + 1) * HALF, :] = (att[0:D] / att[D : D + 1]).T
    return out, res


def kernel(x, Wq, Wk, Wv):
    out, _ = _run(
        np.asarray(x, dtype=np.float32),
        np.asarray(Wq, dtype=np.float32),
        np.asarray(Wk, dtype=np.float32),
        np.asarray(Wv, dtype=np.float32),
    )
    return out

